# revision 41
# baseline (speedup 1.0000x reference)
"""Trainium2 Bass kernel for nn_EncoderLayer_35124242546745 (sparse window attention
encoder layer).

Structure exploited: inds == arange(N), so flat2window/window2flat are identity
maps -- window w, slot s is flat token w*64+s, with slots >= N padding.

Sharding: window/data parallel over 8 cores. W=3125 windows are zero-padded to
3136 = 8*392; each core owns 392 windows = 25088 tokens. All parameters are
replicated. Each core runs an identical (SPMD) program on its shard; outputs are
concatenated on the host. The only masked window (3124: 32 valid tokens, 32
padded key slots) is recomputed exactly on the host and patched in.

V2 changes vs the first working version (engine-balance rework driven by the
TimelineSim cost model):
  - src arrives pre-transposed AND pre-cast to bf16 from the host (srcT), like
    posT: kills the natural-layout src load, the fp32 PE transpose and its
    PSUM evacuation.
  - All activation-engine functions come from one act table
    (natural_log_exp_and_others: Exp/Ln/Identity/Relu/Copy) so the ACT engine
    never reloads function tables (was 2 reloads x 1283ns per block).
  - LN rstd = exp(-0.5 * ln(var + eps)) on ACT instead of sqrt+reciprocal.
  - PSUM evacuations with fused bias (q/k) and bias+relu (FFN) moved to the
    ACT engine; qkin add on DVE bf16; v-tile copies on GPSIMD with the
    ones/zeros padding memset once into persistent buffers.
  - Residual adds are PE matmuls accumulating into PSUM: src via an
    identity-matmul of srcT into the out-projection PSUM; z*ln1_g via a
    diag(ln1_g) matmul of zT into the FFN2 PSUM. LN stats + normalize read
    straight from PSUM; x1/x2 never materialize in SBUF.
  - Softmax normalize fused: stream_shuffle broadcasts denominators from the
    attn@v PSUM, one tensor_tensor multiplies PSUM by the reciprocal while
    evacuating to bf16 SBUF.
"""

from contextlib import ExitStack

import numpy as np
import ml_dtypes

import concourse.bacc as bacc
import concourse.bass as bass
import concourse.tile as tile
from concourse import mybir
from concourse.bass_utils import run_bass_kernel_spmd

BF16 = ml_dtypes.bfloat16

N = 199968
W = 3125
S = 64
D = 128
H = 8
DH = 16
DFF = 256

NCORES = 8
WC = 392                # windows per core (3136 total, 11 zero-pad windows)
TC = WC * S             # 25088 tokens per core
NB = WC // 8            # 49 blocks of 8 windows (512 tokens)
BT = 512                # tokens per block
NVBUF = 4               # persistent padded v-tile buffers

F32 = mybir.dt.float32
BF = mybir.dt.bfloat16
AX = mybir.AluOpType
AF = mybir.ActivationFunctionType


def build_bass(nb=NB, plain_out=False):
    nc = bacc.Bacc("TRN2", target_bir_lowering=False, debug=False,
                   enable_asserts=False, num_devices=1)
    tc_tokens = nb * BT

    srcT_d = nc.dram_tensor("srcT", [D, tc_tokens], BF, kind="ExternalInput")
    posT_d = nc.dram_tensor("posT", [D, tc_tokens], BF, kind="ExternalInput")
    out_d = nc.dram_tensor("out", [tc_tokens, D], F32, kind="ExternalOutput")

    wnames_bf = ["wq_lo_t", "wq_hi_t", "wk_t", "wv_t", "wo_t",
                 "w1_lo_t", "w1_hi_t", "w2_lo_t", "w2_hi_t",
                 "g2rep", "b2rep", "ident_bf", "diag_g1"]
    w_d = {n: nc.dram_tensor(n, [D, D], BF, kind="ExternalInput") for n in wnames_bf}
    for n in ["bq_lo", "bq_hi", "bk", "b1_lo", "b1_hi"]:
        w_d[n] = nc.dram_tensor(n, [D, 1], F32, kind="ExternalInput")
    for n in ["outb_row", "b2b_row"]:
        w_d[n] = nc.dram_tensor(n, [1, D], BF, kind="ExternalInput")

    with tile.TileContext(nc, pool_alloc_mode="queue") as tc, ExitStack() as es:
        consts = es.enter_context(tc.tile_pool(name="consts", bufs=1))
        work = es.enter_context(tc.tile_pool(name="work", bufs=4))
        small = es.enter_context(tc.tile_pool(name="small", bufs=8))
        # PSUM budget (8 banks x 2KB): front pool 2, scores 2, attn@v out 2,
        # back pool 2. Front (q/k/v) and back (x1/zT/h1/x2) pools are split so
        # block b+1's attention never waits on block b's FFN tail.
        mmps = es.enter_context(tc.tile_pool(name="mmps", bufs=2, space="PSUM"))
        resps = es.enter_context(tc.tile_pool(name="resps", bufs=2, space="PSUM"))
        scps = es.enter_context(tc.tile_pool(name="scps", bufs=1, space="PSUM"))
        ops = es.enter_context(tc.tile_pool(name="ops", bufs=1, space="PSUM"))

        # ---- constants ----
        cw = {}
        for n, dr in w_d.items():
            shp = list(dr.shape)
            cw[n] = consts.tile(shp, dr.dtype, tag=n, name=n)
            nc.sync.dma_start(out=cw[n][:], in_=dr[:])
        ones_row = consts.tile([1, D], BF, tag="ones_row")
        nc.vector.memset(ones_row[:], 1.0)
        ones_bt = consts.tile([1, BT], BF, tag="ones_bt")
        nc.vector.memset(ones_bt[:], 1.0)
        eps_t = consts.tile([D, 1], F32, tag="eps")
        nc.vector.memset(eps_t[:], 1e-5)

        # persistent padded v tiles: [D, 4chunk, H, 17]; cols 0:16 written per
        # block, col 16 = ones (gives the softmax denominator as the 17th
        # column of each attn@v output). Ones initialized once.
        v_bufs = []
        for k in range(NVBUF):
            vb = consts.tile([D, 4, H, 17], BF, tag=f"v_buf{k}", name=f"v_buf{k}")
            nc.vector.memset(vb[:, :, :, 16:17], 1.0)
            v_bufs.append(vb)

        def bcast4(t):
            # [128,128] const tile read as [128, 4, 128] (free-dim broadcast)
            a = t[:]
            return bass.AP(tensor=a.tensor, offset=a.offset,
                           ap=[list(a.ap[0]), [0, 4], list(a.ap[1])])

        def emit_front(b):
            t0 = b * BT
            # ---- loads (bf16, pre-transposed on host) ----
            srcT = work.tile([D, BT], BF, tag="srcT", bufs=4)
            nc.sync.dma_start(out=srcT[:], in_=srcT_d[:, t0:t0 + BT])
            posTb = work.tile([D, BT], BF, tag="posTb", bufs=4)
            nc.gpsimd.dma_start(out=posTb[:], in_=posT_d[:, t0:t0 + BT])

            qkinT = work.tile([D, BT], BF, tag="qkinT")
            nc.vector.tensor_tensor(qkinT[:], srcT[:], posTb[:], AX.add)

            # ---- q/k projections (feature-on-partition), bias on ACT ----
            def proj_act(wname, bcol, tag):
                ps = mmps.tile([D, BT], F32, tag="mm")
                nc.tensor.matmul(ps[:], cw[wname][:], qkinT[:])
                sb = work.tile([D, BT], BF, tag=tag)
                nc.scalar.activation(sb[:], ps[:], AF.Identity, bias=cw[bcol][:])
                return sb
            qlo = proj_act("wq_lo_t", "bq_lo", "qlo")
            qhi = proj_act("wq_hi_t", "bq_hi", "qhi")
            kT = proj_act("wk_t", "bk", "kT")

            # ---- v projection (natural layout) into persistent padded tiles ----
            v_ps = mmps.tile([D, 4, D], F32, tag="mm")
            for c in range(4):
                nc.tensor.matmul(v_ps[:, c, :],
                                 srcT[:, c * 128:(c + 1) * 128], cw["wv_t"][:])
            vt = v_bufs[b % NVBUF]
            nc.scalar.activation(
                vt[:, :, :, 0:16],
                v_ps[:].rearrange("p c (h e) -> p c h e", h=H), AF.Copy)

            # ---- attention ----
            # PSUM packing rule (HW-probed): two in-flight matmuls may write
            # the same PSUM bank at different column offsets only from the
            # same (row_grp, col_grp) subarray. Scores: bank = strip (4 banks,
            # cols = pair*128 + qparity*64, rows = window half). attn@v:
            # bank = half, cols = headparity*256 + pair*64, rows = head slot.
            # scores/exp/attn@v in two head-pair rounds (s in {0,1}, {2,3}) so
            # the scores PSUM tile is 2 banks instead of 4.
            # attn@v is computed in NATURAL layout: expS holds S^T per
            # (window, head) with k-tokens on partitions aligned to the
            # window's partition-half, so o[q,(h,e)] = expS_strip.T @ v_strip
            # directly. rhs is [64,17] (16 v dims + ones col -> denominator).
            # o_ps banks are indexed by pp=p//2; emission is half-outer so
            # consecutive in-flight matmuls to one bank share a row group.
            # o_ps: bank pp = p//2 (512-col stride); inside a bank the
            # (p%2, h) output block sits at cols (p%2*8+h)*17 .. +17
            o_ps = ops.tile([D, 2, 512], F32, tag="o_ps")
            for r in range(2):
                sc_ps = scps.tile([D, 2, BT], F32, tag="sc")
                for p in range(4):
                    for si in range(2):
                        s = 2 * r + si
                        for hp in range(2):
                            qsel = qlo if hp == 0 else qhi
                            for half in range(2):
                                wcol = p * 128 + half * 64
                                nc.tensor.matmul(
                                    sc_ps[64 * half:64 * half + 64, si,
                                          p * 128 + hp * 64:p * 128 + hp * 64 + 64],
                                    kT[32 * s:32 * s + 32, wcol:wcol + 64],
                                    qsel[32 * s:32 * s + 32, wcol:wcol + 64],
                                    tile_position=(32 * s, 64 * half))
                expS = work.tile([D, 2 * BT], BF, tag="expS", bufs=4)
                nc.scalar.activation(
                    expS[:].rearrange("p (a b) -> p a b", a=2), sc_ps[:], AF.Exp)
                for half in range(2):
                    for p in range(4):
                        for hh in range(4):
                            h = 4 * r + hh
                            s = h // 2
                            ecol = (s - 2 * r) * 512 + p * 128 + (h % 2) * 64
                            base = ((p % 2) * H + h) * 17
                            nc.tensor.matmul(
                                o_ps[64 * half:64 * half + 64,
                                     p // 2, base:base + 17],
                                expS[64 * half:64 * half + 64, ecol:ecol + 64],
                                vt[64 * half:64 * half + 64, p, h, :],
                                tile_position=(64 * half, 64 * half))

            # ---- softmax normalize in natural layout ----
            # reciprocal of the 32 denominator columns, then one fused
            # multiply-evacuate (PSUM * rcp-broadcast -> bf16 SBUF)
            ob = o_ps[:]  # [part, [pp 2], [1, 512]]
            pdim = list(ob.ap[0])
            den_ap = bass.AP(tensor=ob.tensor, offset=ob.offset + 16,
                             ap=[pdim, [512, 2], [136, 2], [17, H]])
            ov_ap = bass.AP(tensor=ob.tensor, offset=ob.offset,
                            ap=[pdim, [512, 2], [136, 2], [17, H], [1, 16]])
            rcp = small.tile([D, 2, 2, H], BF, tag="rcp")
            with nc.allow_low_precision("softmax denominators are O(64); bf16 recip ok"):
                nc.vector.reciprocal(rcp[:], den_ap)
            o_sb = work.tile([D, 4, H, 16], BF, tag="o_sb")
            rcp_b = rcp[:]
            rcp_bcast = bass.AP(tensor=rcp_b.tensor, offset=rcp_b.offset,
                                ap=[list(rcp_b.ap[0]), list(rcp_b.ap[1]),
                                    list(rcp_b.ap[2]), list(rcp_b.ap[3]),
                                    [0, 16]])
            nc.vector.tensor_tensor(
                o_sb[:].rearrange("p (pp c) h e -> p pp c h e", pp=2),
                ov_ap, rcp_bcast, AX.mult)

            return srcT, o_sb

        def emit_back(b, srcT, o_sb):
            t0 = b * BT
            # ---- transpose o -> oT (PE), evacuate, then out projection +
            # bias + src residual, all in PSUM ----
            oT_ps = mmps.tile([D, BT], BF, tag="mm", name="oT_ps")
            o_sb4 = o_sb[:].rearrange("p c h e -> p c (h e)")
            for c in range(4):
                nc.tensor.transpose(oT_ps[:, c * 128:(c + 1) * 128],
                                    o_sb4[:, c, :], cw["ident_bf"][:])
            oT = work.tile([D, BT], BF, tag="oT")
            nc.vector.tensor_copy(oT[:], oT_ps[:])

            x1_ps = resps.tile([D, 4, D], F32, tag="res")
            for c in range(4):
                nc.tensor.matmul(x1_ps[:, c, :], oT[:, c * 128:(c + 1) * 128],
                                 cw["wo_t"][:], start=True, stop=False)
                nc.tensor.matmul(x1_ps[:, c, :], ones_row[:],
                                 cw["outb_row"][:], start=False, stop=False)
                nc.tensor.matmul(x1_ps[:, c, :], srcT[:, c * 128:(c + 1) * 128],
                                 cw["ident_bf"][:], start=False, stop=True)

            # ---- LN1 from PSUM; rstd = exp(-0.5 ln(var+eps)) on ACT ----
            mv = small.tile([D, 2, 4], F32, tag="mv")
            for c in range(4):
                st = small.tile([D, 6], F32, tag="bnst")
                nc.vector.bn_stats(out=st[:], in_=x1_ps[:, c, :])
                nc.vector.bn_aggr(out=mv[:, :, c], in_=st[:])
            lnv = small.tile([D, 4], F32, tag="lnv")
            nc.scalar.activation(lnv[:], mv[:, 1, :], AF.Ln, bias=eps_t[:])
            rstd = small.tile([D, 4], F32, tag="rstd")
            nc.scalar.activation(rstd[:], lnv[:], AF.Exp, scale=-0.5)
            z = work.tile([D, 4, D], BF, tag="z")
            for c in range(4):
                nc.vector.tensor_scalar(z[:, c, :], x1_ps[:, c, :],
                                        mv[:, 0, c:c + 1], rstd[:, c:c + 1],
                                        AX.subtract, AX.mult)

            # ---- transpose z -> zT (PE), evacuate on ACT ----
            zT_ps = resps.tile([D, BT], BF, tag="res", name="zT_ps")
            for c in range(4):
                nc.tensor.transpose(zT_ps[:, c * 128:(c + 1) * 128],
                                    z[:, c, :], cw["ident_bf"][:])
            zT = work.tile([D, BT], BF, tag="zT")
            nc.scalar.activation(zT[:], zT_ps[:], AF.Copy)

            # ---- FFN (bias+relu fused on ACT) ----
            h1lo_ps = resps.tile([D, BT], F32, tag="res")
            nc.tensor.matmul(h1lo_ps[:], cw["w1_lo_t"][:], zT[:])
            h1lo = work.tile([D, BT], BF, tag="h1lo")
            nc.scalar.activation(h1lo[:], h1lo_ps[:], AF.Relu, bias=cw["b1_lo"][:])
            h1hi_ps = resps.tile([D, BT], F32, tag="res")
            nc.tensor.matmul(h1hi_ps[:], cw["w1_hi_t"][:], zT[:])
            h1hi = work.tile([D, BT], BF, tag="h1hi")
            nc.vector.tensor_scalar(h1hi[:], h1hi_ps[:], cw["b1_hi"][:], 0.0,
                                    AX.add, AX.max)

            # ---- FFN2 + bias + z*g1 residual (diag matmul), x2 in PSUM ----
            x2_ps = resps.tile([D, 4, D], F32, tag="res")
            for c in range(4):
                nc.tensor.matmul(x2_ps[:, c, :], h1lo[:, c * 128:(c + 1) * 128],
                                 cw["w2_lo_t"][:], start=True, stop=False)
                nc.tensor.matmul(x2_ps[:, c, :], h1hi[:, c * 128:(c + 1) * 128],
                                 cw["w2_hi_t"][:], start=False, stop=False)
                nc.tensor.matmul(x2_ps[:, c, :], ones_row[:],
                                 cw["b2b_row"][:], start=False, stop=False)
                nc.tensor.matmul(x2_ps[:, c, :], zT[:, c * 128:(c + 1) * 128],
                                 cw["diag_g1"][:], start=False, stop=True)

            # ---- LN2 from PSUM ----
            mv2 = small.tile([D, 2, 4], F32, tag="mv2")
            for c in range(4):
                st2 = small.tile([D, 6], F32, tag="bnst2")
                nc.vector.bn_stats(out=st2[:], in_=x2_ps[:, c, :])
                nc.vector.bn_aggr(out=mv2[:, :, c], in_=st2[:])
            lnv2 = small.tile([D, 4], F32, tag="lnv2")
            nc.scalar.activation(lnv2[:], mv2[:, 1, :], AF.Ln, bias=eps_t[:])
            rstd2 = small.tile([D, 4], F32, tag="rstd2")
            nc.scalar.activation(rstd2[:], lnv2[:], AF.Exp, scale=-0.5)
            if plain_out:
                # ln2_g == 1, ln2_b == 0 (checked at runtime): the normalize
                # IS the output; write fp32 directly and skip the affine
                outf = work.tile([D, 4, D], F32, tag="outf")
                for c in range(4):
                    nc.vector.tensor_scalar(outf[:, c, :], x2_ps[:, c, :],
                                            mv2[:, 0, c:c + 1], rstd2[:, c:c + 1],
                                            AX.subtract, AX.mult)
            else:
                xh2 = work.tile([D, 4, D], BF, tag="xh2")
                for c in range(4):
                    nc.vector.tensor_scalar(xh2[:, c, :], x2_ps[:, c, :],
                                            mv2[:, 0, c:c + 1], rstd2[:, c:c + 1],
                                            AX.subtract, AX.mult)
                # ---- final affine (per-feature) on GPSIMD ----
                tmo = work.tile([D, 4, D], BF, tag="tmo")
                nc.gpsimd.tensor_tensor(tmo[:], xh2[:], bcast4(cw["g2rep"]), AX.mult)
                outf = work.tile([D, 4, D], F32, tag="outf")
                nc.gpsimd.tensor_tensor(outf[:], tmo[:], bcast4(cw["b2rep"]), AX.add)

            nc.sync.dma_start(
                out=out_d[t0:t0 + BT, :].rearrange("(c p) d -> p c d", p=128),
                in_=outf[:])

        for b in range(nb):
            emit_back(b, *emit_front(b))

    nc.compile()
    _dedupe_act_table_loads(nc)
    return nc


def _dedupe_act_table_loads(nc):
    """All activation functions used here (Exp, Ln, Identity, Copy, Relu) live
    in one act-function table (natural_log_exp_and_others), but the insertion
    pass picks the first matching set per function, alternating tables and
    reloading (1283ns each) several times per block. Retarget every load to
    the shared set and keep only the first load per basic block (the loads
    carry no semaphore ops, so removal is sync-safe)."""
    needed = {"exp", "ln", "identity", "copy", "relu"}
    set_id = 6  # natural_log_exp_and_others in act_info.json
    try:
        from concourse.hw_specs import get_activation_tables
        tables = list(get_activation_tables(nc.m.arch).items())
        for idx, (name, funcs) in enumerate(tables):
            fl = {str(f).split(".")[-1].lower() for f in funcs}
            if needed <= fl:
                set_id = idx
                break
    except Exception:
        pass
    for fn in nc.m.functions:
        for blk in fn.blocks:
            drop = []
            seen = False
            for ins in blk.instructions:
                if isinstance(ins, mybir.InstLoadActFuncSet):
                    assert ins.sync_info is None
                    if seen:
                        drop.append(ins)
                    else:
                        ins.act_func_set_id = set_id
                        seen = True
            for ins in drop:
                blk.instructions.remove(ins)


def prep_weights(in_proj_w, in_proj_b, out_w, out_b, w1, b1, w2, b2,
                 ln1_g, ln1_b, ln2_g, ln2_b):
    Wq, Wk, Wv = in_proj_w[:D], in_proj_w[D:2 * D], in_proj_w[2 * D:]
    bq, bk, bv = in_proj_b[:D], in_proj_b[D:2 * D], in_proj_b[2 * D:]
    scale = 1.0 / np.sqrt(DH)
    Wq = Wq * scale
    bq = bq * scale

    def bf(x):
        return np.ascontiguousarray(x).astype(BF16)

    w = {}
    # zero-interleaved padded q weights: strip s of lo = head 2s in rows
    # [32s,32s+16); strip s of hi = head 2s+1 in rows [32s+16,32s+32)
    A_lo = np.zeros((D, D), np.float32)
    A_hi = np.zeros((D, D), np.float32)
    b_lo = np.zeros((D, 1), np.float32)
    b_hi = np.zeros((D, 1), np.float32)
    for s in range(4):
        A_lo[32 * s:32 * s + 16] = Wq[16 * (2 * s):16 * (2 * s) + 16]
        b_lo[32 * s:32 * s + 16, 0] = bq[16 * (2 * s):16 * (2 * s) + 16]
        A_hi[32 * s + 16:32 * s + 32] = Wq[16 * (2 * s + 1):16 * (2 * s + 1) + 16]
        b_hi[32 * s + 16:32 * s + 32, 0] = bq[16 * (2 * s + 1):16 * (2 * s + 1) + 16]
    w["wq_lo_t"] = bf(A_lo.T)
    w["wq_hi_t"] = bf(A_hi.T)
    w["bq_lo"] = np.ascontiguousarray(b_lo)
    w["bq_hi"] = np.ascontiguousarray(b_hi)
    w["wk_t"] = bf(Wk.T)
    w["bk"] = np.ascontiguousarray(bk.reshape(D, 1)).astype(np.float32)
    w["wv_t"] = bf(Wv.T)

    w["wo_t"] = bf(out_w.T)
    out_b_p = out_b + out_w @ bv  # attn rows sum to 1 -> v bias folds here
    w["outb_row"] = bf(out_b_p.reshape(1, D))

    W1p = w1 * ln1_g[None, :]
    b1p = b1 + w1 @ ln1_b
    w["w1_lo_t"] = bf(W1p[0:128].T)
    w["w1_hi_t"] = bf(W1p[128:256].T)
    w["b1_lo"] = np.ascontiguousarray(b1p[0:128].reshape(D, 1)).astype(np.float32)
    w["b1_hi"] = np.ascontiguousarray(b1p[128:256].reshape(D, 1)).astype(np.float32)
    w["w2_lo_t"] = bf(w2[:, 0:128].T)
    w["w2_hi_t"] = bf(w2[:, 128:256].T)
    w["b2b_row"] = bf((b2 + ln1_b).reshape(1, D))

    w["diag_g1"] = bf(np.diag(ln1_g).astype(np.float32))
    w["g2rep"] = bf(np.broadcast_to(ln2_g, (D, D)))
    w["b2rep"] = bf(np.broadcast_to(ln2_b, (D, D)))
    w["ident_bf"] = bf(np.eye(D, dtype=np.float32))
    return w


_CACHED_NC = {}


def _get_nc(plain_out=True):
    if plain_out not in _CACHED_NC:
        _CACHED_NC[plain_out] = build_bass(NB, plain_out=plain_out)
    return _CACHED_NC[plain_out]


def _host_window_ref(src_w, pos_w, mask_w, in_proj_w, in_proj_b, out_w, out_b,
                     w1, b1, w2, b2, ln1_g, ln1_b, ln2_g, ln2_b):
    """Exact fp32 reference for a single window (used to patch masked tokens)."""
    Wq, Wk, Wv = in_proj_w[:D], in_proj_w[D:2 * D], in_proj_w[2 * D:]
    bq, bk, bv = in_proj_b[:D], in_proj_b[D:2 * D], in_proj_b[2 * D:]
    qk_in = src_w + pos_w
    q = qk_in @ Wq.T + bq
    k = qk_in @ Wk.T + bk
    v = src_w @ Wv.T + bv
    qh = q.reshape(S, H, DH)
    kh = k.reshape(S, H, DH)
    vh = v.reshape(S, H, DH)
    sc = np.einsum("qhd,khd->hqk", qh, kh) / np.sqrt(DH)
    sc = np.where(mask_w[None, None, :], -np.inf, sc)
    sc = sc - sc.max(-1, keepdims=True)
    e = np.exp(sc)
    attn = e / e.sum(-1, keepdims=True)
    o = np.einsum("hqk,khd->qhd", attn, vh).reshape(S, D)
    o = o @ out_w.T + out_b
    x = src_w + o
    mu = x.mean(-1, keepdims=True)
    va = ((x - mu) ** 2).mean(-1, keepdims=True)
    x = (x - mu) / np.sqrt(va + 1e-5) * ln1_g + ln1_b
    ffn = np.maximum(x @ w1.T + b1, 0.0) @ w2.T + b2
    x2 = x + ffn
    mu2 = x2.mean(-1, keepdims=True)
    va2 = ((x2 - mu2) ** 2).mean(-1, keepdims=True)
    return (x2 - mu2) / np.sqrt(va2 + 1e-5) * ln2_g + ln2_b


def kernel(src, pos, inds, key_padding_mask, in_proj_w, in_proj_b,
           out_w, out_b, w1, b1, w2, b2, ln1_g, ln1_b, ln2_g, ln2_b):
    src = np.asarray(src, np.float32)
    pos = np.asarray(pos, np.float32)
    args = dict(in_proj_w=np.asarray(in_proj_w, np.float32),
                in_proj_b=np.asarray(in_proj_b, np.float32),
                out_w=np.asarray(out_w, np.float32),
                out_b=np.asarray(out_b, np.float32),
                w1=np.asarray(w1, np.float32), b1=np.asarray(b1, np.float32),
                w2=np.asarray(w2, np.float32), b2=np.asarray(b2, np.float32),
                ln1_g=np.asarray(ln1_g, np.float32),
                ln1_b=np.asarray(ln1_b, np.float32),
                ln2_g=np.asarray(ln2_g, np.float32),
                ln2_b=np.asarray(ln2_b, np.float32))
    wts = prep_weights(**args)

    # zero-pad to 3136 windows, transpose + cast to bf16, shard columns
    total = NCORES * TC
    srcT_full = np.zeros((D, total), BF16)
    srcT_full[:, :N] = src.T.astype(BF16)
    posT_full = np.zeros((D, total), BF16)
    posT_full[:, :W * S] = pos.reshape(W * S, D).T.astype(BF16)

    in_maps = []
    for c in range(NCORES):
        lo, hi = c * TC, (c + 1) * TC
        m = {"srcT": np.ascontiguousarray(srcT_full[:, lo:hi]),
             "posT": np.ascontiguousarray(posT_full[:, lo:hi])}
        m.update(wts)
        in_maps.append(m)

    plain_out = bool(np.allclose(args["ln2_g"], 1.0) and
                     np.allclose(args["ln2_b"], 0.0))
    nc = _get_nc(plain_out)
    res = run_bass_kernel_spmd(nc, in_maps, list(range(NCORES)))
    out = np.concatenate([res.results[c]["out"] for c in range(NCORES)], axis=0)
    out = out[:N].astype(np.float32)

    # patch the one masked window (3124: tokens 199936..199968) exactly
    wlast = N // S  # 3124
    t0 = wlast * S
    nvalid = N - t0
    src_w = np.zeros((S, D), np.float32)
    src_w[:nvalid] = src[t0:N]
    mask_w = np.asarray(key_padding_mask)[wlast]
    patched = _host_window_ref(src_w, pos[wlast], mask_w, **args)
    out[t0:N] = patched[:nvalid]
    return out


# revision 42
# speedup vs baseline: 1.0310x; 1.0310x over previous
"""Trainium2 Bass kernel for nn_EncoderLayer_35124242546745 (sparse window attention
encoder layer).

Structure exploited: inds == arange(N), so flat2window/window2flat are identity
maps -- window w, slot s is flat token w*64+s, with slots >= N padding.

Sharding: window/data parallel over 8 cores. W=3125 windows are zero-padded to
3136 = 8*392; each core owns 392 windows = 25088 tokens. All parameters are
replicated. Each core runs an identical (SPMD) program on its shard; outputs are
concatenated on the host. The only masked window (3124: 32 valid tokens, 32
padded key slots) is recomputed exactly on the host and patched in.

V2 changes vs the first working version (engine-balance rework driven by the
TimelineSim cost model):
  - src arrives pre-transposed AND pre-cast to bf16 from the host (srcT), like
    posT: kills the natural-layout src load, the fp32 PE transpose and its
    PSUM evacuation.
  - All activation-engine functions come from one act table
    (natural_log_exp_and_others: Exp/Ln/Identity/Relu/Copy) so the ACT engine
    never reloads function tables (was 2 reloads x 1283ns per block).
  - LN rstd = exp(-0.5 * ln(var + eps)) on ACT instead of sqrt+reciprocal.
  - PSUM evacuations with fused bias (q/k) and bias+relu (FFN) moved to the
    ACT engine; qkin add on DVE bf16; v-tile copies on GPSIMD with the
    ones/zeros padding memset once into persistent buffers.
  - Residual adds are PE matmuls accumulating into PSUM: src via an
    identity-matmul of srcT into the out-projection PSUM; z*ln1_g via a
    diag(ln1_g) matmul of zT into the FFN2 PSUM. LN stats + normalize read
    straight from PSUM; x1/x2 never materialize in SBUF.
  - Softmax normalize fused: stream_shuffle broadcasts denominators from the
    attn@v PSUM, one tensor_tensor multiplies PSUM by the reciprocal while
    evacuating to bf16 SBUF.
"""

from contextlib import ExitStack

import numpy as np
import ml_dtypes

import concourse.bacc as bacc
import concourse.bass as bass
import concourse.tile as tile
from concourse import mybir
from concourse.bass_utils import run_bass_kernel_spmd

BF16 = ml_dtypes.bfloat16

N = 199968
W = 3125
S = 64
D = 128
H = 8
DH = 16
DFF = 256

NCORES = 8
WC = 392                # windows per core (3136 total, 11 zero-pad windows)
TC = WC * S             # 25088 tokens per core
NB = WC // 8            # 49 blocks of 8 windows (512 tokens)
BT = 512                # tokens per block
NVBUF = 4               # persistent padded v-tile buffers

F32 = mybir.dt.float32
BF = mybir.dt.bfloat16
AX = mybir.AluOpType
AF = mybir.ActivationFunctionType


def build_bass(nb=NB, plain_out=False):
    nc = bacc.Bacc("TRN2", target_bir_lowering=False, debug=False,
                   enable_asserts=False, num_devices=1)
    tc_tokens = nb * BT

    srcT_d = nc.dram_tensor("srcT", [D, tc_tokens], BF, kind="ExternalInput")
    posT_d = nc.dram_tensor("posT", [D, tc_tokens], BF, kind="ExternalInput")
    out_d = nc.dram_tensor("out", [tc_tokens, D], F32, kind="ExternalOutput")

    wnames_bf = ["wq_lo_t", "wq_hi_t", "wk_t", "wv_t", "wo_t",
                 "w1_lo_t", "w1_hi_t", "w2_lo_t", "w2_hi_t",
                 "g2rep", "b2rep", "ident_bf", "diag_g1"]
    w_d = {n: nc.dram_tensor(n, [D, D], BF, kind="ExternalInput") for n in wnames_bf}
    for n in ["bq_lo", "bq_hi", "bk", "b1_lo", "b1_hi"]:
        w_d[n] = nc.dram_tensor(n, [D, 1], F32, kind="ExternalInput")
    for n in ["outb_row", "b2b_row"]:
        w_d[n] = nc.dram_tensor(n, [1, D], BF, kind="ExternalInput")

    with tile.TileContext(nc, pool_alloc_mode="queue") as tc, ExitStack() as es:
        consts = es.enter_context(tc.tile_pool(name="consts", bufs=1))
        work = es.enter_context(tc.tile_pool(name="work", bufs=4))
        small = es.enter_context(tc.tile_pool(name="small", bufs=8))
        # PSUM budget (8 banks x 2KB): front pool 2, scores 2, attn@v out 2,
        # back pool 2. Front (q/k/v) and back (x1/zT/h1/x2) pools are split so
        # block b+1's attention never waits on block b's FFN tail.
        mmps = es.enter_context(tc.tile_pool(name="mmps", bufs=2, space="PSUM"))
        resps = es.enter_context(tc.tile_pool(name="resps", bufs=2, space="PSUM"))
        scps = es.enter_context(tc.tile_pool(name="scps", bufs=1, space="PSUM"))
        ops = es.enter_context(tc.tile_pool(name="ops", bufs=1, space="PSUM"))

        # ---- constants ----
        cw = {}
        for n, dr in w_d.items():
            shp = list(dr.shape)
            cw[n] = consts.tile(shp, dr.dtype, tag=n, name=n)
            nc.sync.dma_start(out=cw[n][:], in_=dr[:])
        ones_row = consts.tile([1, D], BF, tag="ones_row")
        nc.vector.memset(ones_row[:], 1.0)
        ones_bt = consts.tile([1, BT], BF, tag="ones_bt")
        nc.vector.memset(ones_bt[:], 1.0)
        eps_t = consts.tile([D, 1], F32, tag="eps")
        nc.vector.memset(eps_t[:], 1e-5)

        # persistent padded v tiles: [D, 4chunk, H, 17]; cols 0:16 written per
        # block, col 16 = ones (gives the softmax denominator as the 17th
        # column of each attn@v output). Ones initialized once.
        v_bufs = []
        for k in range(NVBUF):
            vb = consts.tile([D, 4, H, 17], BF, tag=f"v_buf{k}", name=f"v_buf{k}")
            nc.vector.memset(vb[:, :, :, 16:17], 1.0)
            v_bufs.append(vb)

        def bcast4(t):
            # [128,128] const tile read as [128, 4, 128] (free-dim broadcast)
            a = t[:]
            return bass.AP(tensor=a.tensor, offset=a.offset,
                           ap=[list(a.ap[0]), [0, 4], list(a.ap[1])])

        def emit_front(b):
            t0 = b * BT
            # ---- loads (bf16, pre-transposed on host) ----
            srcT = work.tile([D, BT], BF, tag="srcT", bufs=4)
            nc.sync.dma_start(out=srcT[:], in_=srcT_d[:, t0:t0 + BT])
            posTb = work.tile([D, BT], BF, tag="posTb", bufs=4)
            nc.gpsimd.dma_start(out=posTb[:], in_=posT_d[:, t0:t0 + BT])

            qkinT = work.tile([D, BT], BF, tag="qkinT")
            nc.vector.tensor_tensor(qkinT[:], srcT[:], posTb[:], AX.add)

            # ---- q/k projections (feature-on-partition), bias on ACT ----
            def proj_act(wname, bcol, tag):
                ps = mmps.tile([D, BT], F32, tag="mm")
                nc.tensor.matmul(ps[:], cw[wname][:], qkinT[:])
                sb = work.tile([D, BT], BF, tag=tag)
                nc.scalar.activation(sb[:], ps[:], AF.Identity, bias=cw[bcol][:])
                return sb
            qlo = proj_act("wq_lo_t", "bq_lo", "qlo")
            qhi = proj_act("wq_hi_t", "bq_hi", "qhi")
            kT = proj_act("wk_t", "bk", "kT")

            # ---- v projection (natural layout) into persistent padded tiles ----
            v_ps = mmps.tile([D, 4, D], F32, tag="mm")
            for c in range(4):
                nc.tensor.matmul(v_ps[:, c, :],
                                 srcT[:, c * 128:(c + 1) * 128], cw["wv_t"][:])
            vt = v_bufs[b % NVBUF]
            nc.vector.tensor_copy(
                vt[:, :, :, 0:16],
                v_ps[:].rearrange("p c (h e) -> p c h e", h=H))

            # ---- attention ----
            # PSUM packing rule (HW-probed): two in-flight matmuls may write
            # the same PSUM bank at different column offsets only from the
            # same (row_grp, col_grp) subarray. Scores: bank = strip (4 banks,
            # cols = pair*128 + qparity*64, rows = window half). attn@v:
            # bank = half, cols = headparity*256 + pair*64, rows = head slot.
            # scores/exp/attn@v in two head-pair rounds (s in {0,1}, {2,3}) so
            # the scores PSUM tile is 2 banks instead of 4.
            # attn@v is computed in NATURAL layout: expS holds S^T per
            # (window, head) with k-tokens on partitions aligned to the
            # window's partition-half, so o[q,(h,e)] = expS_strip.T @ v_strip
            # directly. rhs is [64,17] (16 v dims + ones col -> denominator).
            # o_ps banks are indexed by pp=p//2; emission is half-outer so
            # consecutive in-flight matmuls to one bank share a row group.
            # o_ps: bank pp = p//2 (512-col stride); inside a bank the
            # (p%2, h) output block sits at cols (p%2*8+h)*17 .. +17
            o_ps = ops.tile([D, 2, 512], F32, tag="o_ps")
            for r in range(2):
                sc_ps = scps.tile([D, 2, BT], F32, tag="sc")
                for p in range(4):
                    for si in range(2):
                        s = 2 * r + si
                        for hp in range(2):
                            qsel = qlo if hp == 0 else qhi
                            for half in range(2):
                                wcol = p * 128 + half * 64
                                nc.tensor.matmul(
                                    sc_ps[64 * half:64 * half + 64, si,
                                          p * 128 + hp * 64:p * 128 + hp * 64 + 64],
                                    kT[32 * s:32 * s + 32, wcol:wcol + 64],
                                    qsel[32 * s:32 * s + 32, wcol:wcol + 64],
                                    tile_position=(32 * s, 64 * half))
                expS = work.tile([D, 2 * BT], BF, tag="expS", bufs=4)
                nc.scalar.activation(
                    expS[:].rearrange("p (a b) -> p a b", a=2), sc_ps[:], AF.Exp)
                for half in range(2):
                    for p in range(4):
                        for hh in range(4):
                            h = 4 * r + hh
                            s = h // 2
                            ecol = (s - 2 * r) * 512 + p * 128 + (h % 2) * 64
                            base = ((p % 2) * H + h) * 17
                            nc.tensor.matmul(
                                o_ps[64 * half:64 * half + 64,
                                     p // 2, base:base + 17],
                                expS[64 * half:64 * half + 64, ecol:ecol + 64],
                                vt[64 * half:64 * half + 64, p, h, :],
                                tile_position=(64 * half, 64 * half))

            # ---- softmax normalize in natural layout ----
            # reciprocal of the 32 denominator columns, then one fused
            # multiply-evacuate (PSUM * rcp-broadcast -> bf16 SBUF)
            ob = o_ps[:]  # [part, [pp 2], [1, 512]]
            pdim = list(ob.ap[0])
            den_ap = bass.AP(tensor=ob.tensor, offset=ob.offset + 16,
                             ap=[pdim, [512, 2], [136, 2], [17, H]])
            ov_ap = bass.AP(tensor=ob.tensor, offset=ob.offset,
                            ap=[pdim, [512, 2], [136, 2], [17, H], [1, 16]])
            rcp = small.tile([D, 2, 2, H], BF, tag="rcp")
            with nc.allow_low_precision("softmax denominators are O(64); bf16 recip ok"):
                nc.vector.reciprocal(rcp[:], den_ap)
            o_sb = work.tile([D, 4, H, 16], BF, tag="o_sb")
            rcp_b = rcp[:]
            rcp_bcast = bass.AP(tensor=rcp_b.tensor, offset=rcp_b.offset,
                                ap=[list(rcp_b.ap[0]), list(rcp_b.ap[1]),
                                    list(rcp_b.ap[2]), list(rcp_b.ap[3]),
                                    [0, 16]])
            nc.vector.tensor_tensor(
                o_sb[:].rearrange("p (pp c) h e -> p pp c h e", pp=2),
                ov_ap, rcp_bcast, AX.mult)

            return srcT, o_sb

        def emit_back(b, srcT, o_sb):
            t0 = b * BT
            # ---- transpose o -> oT (PE), evacuate, then out projection +
            # bias + src residual, all in PSUM ----
            oT_ps = mmps.tile([D, BT], BF, tag="mm", name="oT_ps")
            o_sb4 = o_sb[:].rearrange("p c h e -> p c (h e)")
            for c in range(4):
                nc.tensor.transpose(oT_ps[:, c * 128:(c + 1) * 128],
                                    o_sb4[:, c, :], cw["ident_bf"][:])
            oT = work.tile([D, BT], BF, tag="oT")
            nc.vector.tensor_copy(oT[:], oT_ps[:])

            x1_ps = resps.tile([D, 4, D], F32, tag="res")
            for c in range(4):
                nc.tensor.matmul(x1_ps[:, c, :], oT[:, c * 128:(c + 1) * 128],
                                 cw["wo_t"][:], start=True, stop=False)
                nc.tensor.matmul(x1_ps[:, c, :], ones_row[:],
                                 cw["outb_row"][:], start=False, stop=False)
                nc.tensor.matmul(x1_ps[:, c, :], srcT[:, c * 128:(c + 1) * 128],
                                 cw["ident_bf"][:], start=False, stop=True)

            # ---- LN1 from PSUM; rstd = exp(-0.5 ln(var+eps)) on ACT ----
            mv = small.tile([D, 2, 4], F32, tag="mv")
            for c in range(4):
                st = small.tile([D, 6], F32, tag="bnst")
                nc.vector.bn_stats(out=st[:], in_=x1_ps[:, c, :])
                nc.vector.bn_aggr(out=mv[:, :, c], in_=st[:])
            lnv = small.tile([D, 4], F32, tag="lnv")
            nc.scalar.activation(lnv[:], mv[:, 1, :], AF.Ln, bias=eps_t[:])
            rstd = small.tile([D, 4], F32, tag="rstd")
            nc.scalar.activation(rstd[:], lnv[:], AF.Exp, scale=-0.5)
            z = work.tile([D, 4, D], BF, tag="z")
            for c in range(4):
                nc.vector.tensor_scalar(z[:, c, :], x1_ps[:, c, :],
                                        mv[:, 0, c:c + 1], rstd[:, c:c + 1],
                                        AX.subtract, AX.mult)

            # ---- transpose z -> zT (PE), evacuate on ACT ----
            zT_ps = resps.tile([D, BT], BF, tag="res", name="zT_ps")
            for c in range(4):
                nc.tensor.transpose(zT_ps[:, c * 128:(c + 1) * 128],
                                    z[:, c, :], cw["ident_bf"][:])
            zT = work.tile([D, BT], BF, tag="zT")
            nc.scalar.activation(zT[:], zT_ps[:], AF.Copy)

            # ---- FFN (bias+relu fused on ACT) ----
            h1lo_ps = resps.tile([D, BT], F32, tag="res")
            nc.tensor.matmul(h1lo_ps[:], cw["w1_lo_t"][:], zT[:])
            h1lo = work.tile([D, BT], BF, tag="h1lo")
            nc.scalar.activation(h1lo[:], h1lo_ps[:], AF.Relu, bias=cw["b1_lo"][:])
            h1hi_ps = resps.tile([D, BT], F32, tag="res")
            nc.tensor.matmul(h1hi_ps[:], cw["w1_hi_t"][:], zT[:])
            h1hi = work.tile([D, BT], BF, tag="h1hi")
            nc.vector.tensor_scalar(h1hi[:], h1hi_ps[:], cw["b1_hi"][:], 0.0,
                                    AX.add, AX.max)

            # ---- FFN2 + bias + z*g1 residual (diag matmul), x2 in PSUM ----
            x2_ps = resps.tile([D, 4, D], F32, tag="res")
            for c in range(4):
                nc.tensor.matmul(x2_ps[:, c, :], h1lo[:, c * 128:(c + 1) * 128],
                                 cw["w2_lo_t"][:], start=True, stop=False)
                nc.tensor.matmul(x2_ps[:, c, :], h1hi[:, c * 128:(c + 1) * 128],
                                 cw["w2_hi_t"][:], start=False, stop=False)
                nc.tensor.matmul(x2_ps[:, c, :], ones_row[:],
                                 cw["b2b_row"][:], start=False, stop=False)
                nc.tensor.matmul(x2_ps[:, c, :], zT[:, c * 128:(c + 1) * 128],
                                 cw["diag_g1"][:], start=False, stop=True)

            # ---- LN2 from PSUM ----
            mv2 = small.tile([D, 2, 4], F32, tag="mv2")
            for c in range(4):
                st2 = small.tile([D, 6], F32, tag="bnst2")
                nc.vector.bn_stats(out=st2[:], in_=x2_ps[:, c, :])
                nc.vector.bn_aggr(out=mv2[:, :, c], in_=st2[:])
            lnv2 = small.tile([D, 4], F32, tag="lnv2")
            nc.scalar.activation(lnv2[:], mv2[:, 1, :], AF.Ln, bias=eps_t[:])
            rstd2 = small.tile([D, 4], F32, tag="rstd2")
            nc.scalar.activation(rstd2[:], lnv2[:], AF.Exp, scale=-0.5)
            if plain_out:
                # ln2_g == 1, ln2_b == 0 (checked at runtime): the normalize
                # IS the output; write fp32 directly and skip the affine
                outf = work.tile([D, 4, D], F32, tag="outf")
                for c in range(4):
                    nc.vector.tensor_scalar(outf[:, c, :], x2_ps[:, c, :],
                                            mv2[:, 0, c:c + 1], rstd2[:, c:c + 1],
                                            AX.subtract, AX.mult)
            else:
                xh2 = work.tile([D, 4, D], BF, tag="xh2")
                for c in range(4):
                    nc.vector.tensor_scalar(xh2[:, c, :], x2_ps[:, c, :],
                                            mv2[:, 0, c:c + 1], rstd2[:, c:c + 1],
                                            AX.subtract, AX.mult)
                # ---- final affine (per-feature) on GPSIMD ----
                tmo = work.tile([D, 4, D], BF, tag="tmo")
                nc.gpsimd.tensor_tensor(tmo[:], xh2[:], bcast4(cw["g2rep"]), AX.mult)
                outf = work.tile([D, 4, D], F32, tag="outf")
                nc.gpsimd.tensor_tensor(outf[:], tmo[:], bcast4(cw["b2rep"]), AX.add)

            nc.sync.dma_start(
                out=out_d[t0:t0 + BT, :].rearrange("(c p) d -> p c d", p=128),
                in_=outf[:])

        for b in range(nb):
            emit_back(b, *emit_front(b))

    nc.compile()
    _dedupe_act_table_loads(nc)
    return nc


def _dedupe_act_table_loads(nc):
    """All activation functions used here (Exp, Ln, Identity, Copy, Relu) live
    in one act-function table (natural_log_exp_and_others), but the insertion
    pass picks the first matching set per function, alternating tables and
    reloading (1283ns each) several times per block. Retarget every load to
    the shared set and keep only the first load per basic block (the loads
    carry no semaphore ops, so removal is sync-safe)."""
    needed = {"exp", "ln", "identity", "copy", "relu"}
    set_id = 6  # natural_log_exp_and_others in act_info.json
    try:
        from concourse.hw_specs import get_activation_tables
        tables = list(get_activation_tables(nc.m.arch).items())
        for idx, (name, funcs) in enumerate(tables):
            fl = {str(f).split(".")[-1].lower() for f in funcs}
            if needed <= fl:
                set_id = idx
                break
    except Exception:
        pass
    for fn in nc.m.functions:
        for blk in fn.blocks:
            drop = []
            seen = False
            for ins in blk.instructions:
                if isinstance(ins, mybir.InstLoadActFuncSet):
                    assert ins.sync_info is None
                    if seen:
                        drop.append(ins)
                    else:
                        ins.act_func_set_id = set_id
                        seen = True
            for ins in drop:
                blk.instructions.remove(ins)


def prep_weights(in_proj_w, in_proj_b, out_w, out_b, w1, b1, w2, b2,
                 ln1_g, ln1_b, ln2_g, ln2_b):
    Wq, Wk, Wv = in_proj_w[:D], in_proj_w[D:2 * D], in_proj_w[2 * D:]
    bq, bk, bv = in_proj_b[:D], in_proj_b[D:2 * D], in_proj_b[2 * D:]
    scale = 1.0 / np.sqrt(DH)
    Wq = Wq * scale
    bq = bq * scale

    def bf(x):
        return np.ascontiguousarray(x).astype(BF16)

    w = {}
    # zero-interleaved padded q weights: strip s of lo = head 2s in rows
    # [32s,32s+16); strip s of hi = head 2s+1 in rows [32s+16,32s+32)
    A_lo = np.zeros((D, D), np.float32)
    A_hi = np.zeros((D, D), np.float32)
    b_lo = np.zeros((D, 1), np.float32)
    b_hi = np.zeros((D, 1), np.float32)
    for s in range(4):
        A_lo[32 * s:32 * s + 16] = Wq[16 * (2 * s):16 * (2 * s) + 16]
        b_lo[32 * s:32 * s + 16, 0] = bq[16 * (2 * s):16 * (2 * s) + 16]
        A_hi[32 * s + 16:32 * s + 32] = Wq[16 * (2 * s + 1):16 * (2 * s + 1) + 16]
        b_hi[32 * s + 16:32 * s + 32, 0] = bq[16 * (2 * s + 1):16 * (2 * s + 1) + 16]
    w["wq_lo_t"] = bf(A_lo.T)
    w["wq_hi_t"] = bf(A_hi.T)
    w["bq_lo"] = np.ascontiguousarray(b_lo)
    w["bq_hi"] = np.ascontiguousarray(b_hi)
    w["wk_t"] = bf(Wk.T)
    w["bk"] = np.ascontiguousarray(bk.reshape(D, 1)).astype(np.float32)
    w["wv_t"] = bf(Wv.T)

    w["wo_t"] = bf(out_w.T)
    out_b_p = out_b + out_w @ bv  # attn rows sum to 1 -> v bias folds here
    w["outb_row"] = bf(out_b_p.reshape(1, D))

    W1p = w1 * ln1_g[None, :]
    b1p = b1 + w1 @ ln1_b
    w["w1_lo_t"] = bf(W1p[0:128].T)
    w["w1_hi_t"] = bf(W1p[128:256].T)
    w["b1_lo"] = np.ascontiguousarray(b1p[0:128].reshape(D, 1)).astype(np.float32)
    w["b1_hi"] = np.ascontiguousarray(b1p[128:256].reshape(D, 1)).astype(np.float32)
    w["w2_lo_t"] = bf(w2[:, 0:128].T)
    w["w2_hi_t"] = bf(w2[:, 128:256].T)
    w["b2b_row"] = bf((b2 + ln1_b).reshape(1, D))

    w["diag_g1"] = bf(np.diag(ln1_g).astype(np.float32))
    w["g2rep"] = bf(np.broadcast_to(ln2_g, (D, D)))
    w["b2rep"] = bf(np.broadcast_to(ln2_b, (D, D)))
    w["ident_bf"] = bf(np.eye(D, dtype=np.float32))
    return w


_CACHED_NC = {}


def _get_nc(plain_out=True):
    if plain_out not in _CACHED_NC:
        _CACHED_NC[plain_out] = build_bass(NB, plain_out=plain_out)
    return _CACHED_NC[plain_out]


def _host_window_ref(src_w, pos_w, mask_w, in_proj_w, in_proj_b, out_w, out_b,
                     w1, b1, w2, b2, ln1_g, ln1_b, ln2_g, ln2_b):
    """Exact fp32 reference for a single window (used to patch masked tokens)."""
    Wq, Wk, Wv = in_proj_w[:D], in_proj_w[D:2 * D], in_proj_w[2 * D:]
    bq, bk, bv = in_proj_b[:D], in_proj_b[D:2 * D], in_proj_b[2 * D:]
    qk_in = src_w + pos_w
    q = qk_in @ Wq.T + bq
    k = qk_in @ Wk.T + bk
    v = src_w @ Wv.T + bv
    qh = q.reshape(S, H, DH)
    kh = k.reshape(S, H, DH)
    vh = v.reshape(S, H, DH)
    sc = np.einsum("qhd,khd->hqk", qh, kh) / np.sqrt(DH)
    sc = np.where(mask_w[None, None, :], -np.inf, sc)
    sc = sc - sc.max(-1, keepdims=True)
    e = np.exp(sc)
    attn = e / e.sum(-1, keepdims=True)
    o = np.einsum("hqk,khd->qhd", attn, vh).reshape(S, D)
    o = o @ out_w.T + out_b
    x = src_w + o
    mu = x.mean(-1, keepdims=True)
    va = ((x - mu) ** 2).mean(-1, keepdims=True)
    x = (x - mu) / np.sqrt(va + 1e-5) * ln1_g + ln1_b
    ffn = np.maximum(x @ w1.T + b1, 0.0) @ w2.T + b2
    x2 = x + ffn
    mu2 = x2.mean(-1, keepdims=True)
    va2 = ((x2 - mu2) ** 2).mean(-1, keepdims=True)
    return (x2 - mu2) / np.sqrt(va2 + 1e-5) * ln2_g + ln2_b


def kernel(src, pos, inds, key_padding_mask, in_proj_w, in_proj_b,
           out_w, out_b, w1, b1, w2, b2, ln1_g, ln1_b, ln2_g, ln2_b):
    src = np.asarray(src, np.float32)
    pos = np.asarray(pos, np.float32)
    args = dict(in_proj_w=np.asarray(in_proj_w, np.float32),
                in_proj_b=np.asarray(in_proj_b, np.float32),
                out_w=np.asarray(out_w, np.float32),
                out_b=np.asarray(out_b, np.float32),
                w1=np.asarray(w1, np.float32), b1=np.asarray(b1, np.float32),
                w2=np.asarray(w2, np.float32), b2=np.asarray(b2, np.float32),
                ln1_g=np.asarray(ln1_g, np.float32),
                ln1_b=np.asarray(ln1_b, np.float32),
                ln2_g=np.asarray(ln2_g, np.float32),
                ln2_b=np.asarray(ln2_b, np.float32))
    wts = prep_weights(**args)

    # zero-pad to 3136 windows, transpose + cast to bf16, shard columns
    total = NCORES * TC
    srcT_full = np.zeros((D, total), BF16)
    srcT_full[:, :N] = src.T.astype(BF16)
    posT_full = np.zeros((D, total), BF16)
    posT_full[:, :W * S] = pos.reshape(W * S, D).T.astype(BF16)

    in_maps = []
    for c in range(NCORES):
        lo, hi = c * TC, (c + 1) * TC
        m = {"srcT": np.ascontiguousarray(srcT_full[:, lo:hi]),
             "posT": np.ascontiguousarray(posT_full[:, lo:hi])}
        m.update(wts)
        in_maps.append(m)

    plain_out = bool(np.allclose(args["ln2_g"], 1.0) and
                     np.allclose(args["ln2_b"], 0.0))
    nc = _get_nc(plain_out)
    res = run_bass_kernel_spmd(nc, in_maps, list(range(NCORES)))
    out = np.concatenate([res.results[c]["out"] for c in range(NCORES)], axis=0)
    out = out[:N].astype(np.float32)

    # patch the one masked window (3124: tokens 199936..199968) exactly
    wlast = N // S  # 3124
    t0 = wlast * S
    nvalid = N - t0
    src_w = np.zeros((S, D), np.float32)
    src_w[:nvalid] = src[t0:N]
    mask_w = np.asarray(key_padding_mask)[wlast]
    patched = _host_window_ref(src_w, pos[wlast], mask_w, **args)
    out[t0:N] = patched[:nvalid]
    return out
